# revision 2
# baseline (speedup 1.0000x reference)
"""Trainium2 Bass kernel v2 for nn_CenterLossN.

Math (per batch n, class c; H=W=384, C=11, N=32):
    res[n,c]   = x[n,c]^2 + centers[n,c]^2 - 2 * x[n,c] @ centers[n,c]
    out[n,h,w] = 1 / sum_c exp(res_c - max_c res_c)
    loss       = sum(clip(out * labels, 1e-12, 1e12)) / (N*H*W)

v2 strategy (data-parallel over N across 8 cores, 4 batches/core):
  - fp8(e4m3) matmul in DoubleRow perf mode: per (plane, mc row-chunk) the
    contraction runs as 2 matmuls over 4 k-tiles: 3 tiles of (-2x)^T @ c
    plus a 4th tile identity @ ee injecting x^2+c^2 into PSUM. 768 PE
    cycles per chunk vs 1536 for the bf16 baseline.
  - host ships xt2e = [3 k-slabs of (-2x)^T | identity slab] fp8 and
    ccee = [3 k-slabs of centers | 3 mc-slabs of ee] fp8, partition-major.
  - PSUM tile [128,3,512] f32 (3 banks) per plane; matmuls write bank-
    aligned [:, mc, :384]; ONE strided drain per plane (ACT/DVE alternate).
  - max/sum over classes via single tensor_reduce on a permuted AP
    (class axis strided-innermost), not 5-op trees.
  - per-(n,mc) tail chain max->sub->exp->sum->recip->stt is emitted
    interleaved into the NEXT group's plane loop (software pipelining).
  clip: only label==0 hits the 1e-12 floor; host adds 1e-12*count exactly.
"""

import numpy as np
import ml_dtypes

N, C, H, W = 32, 11, 384, 384
N_CORES = 8
N_LOC = N // N_CORES          # 4 batches per core
PAIRS = N_LOC * C             # 44 planes per core
MC = H // 128                 # 3 row-chunks
KC = W // 128                 # 3 contraction chunks

# feature flags resolved by optest on this deployment
RECIP_FAST = True     # vector.reciprocal_approx_fast works?
GPSIMD_OK = False     # gpsimd tensor ops work?

_BF16 = ml_dtypes.bfloat16
_FP8 = ml_dtypes.float8_e4m3
_COMPILED = None


def _build(n_loc=N_LOC):
    from contextlib import ExitStack
    import concourse.bass as bass
    import concourse.bacc as bacc
    import concourse.tile as tile
    from concourse import mybir

    bf16 = mybir.dt.bfloat16
    f32 = mybir.dt.float32
    fp8 = mybir.dt.float8e4
    AF = mybir.ActivationFunctionType
    DR = mybir.MatmulPerfMode.DoubleRow

    nc = bacc.Bacc("TRN2", target_bir_lowering=False, debug=False)

    pairs = n_loc * C
    xt2e_d = nc.dram_tensor("xt2e", [pairs, 128, 4, 384], fp8, kind="ExternalInput")
    ccee_d = nc.dram_tensor("ccee", [pairs, 128, 6, 384], fp8, kind="ExternalInput")
    lab_d = nc.dram_tensor("lab", [n_loc, H, W], bf16, kind="ExternalInput")
    out_d = nc.dram_tensor("out", [128, 1], f32, kind="ExternalOutput")

    with ExitStack() as ctx:
        tc = ctx.enter_context(tile.TileContext(nc))
        lx = ctx.enter_context(tc.tile_pool(name="lx", bufs=12))
        lc = ctx.enter_context(tc.tile_pool(name="lc", bufs=12))
        spool2 = ctx.enter_context(tc.tile_pool(name="spool2", bufs=5))
        labp = ctx.enter_context(tc.tile_pool(name="labp", bufs=2))
        tree = ctx.enter_context(tc.tile_pool(name="tree", bufs=2))
        mpool = ctx.enter_context(tc.tile_pool(name="mpool", bufs=4))
        wpool = ctx.enter_context(tc.tile_pool(name="wpool", bufs=2))
        accp = ctx.enter_context(tc.tile_pool(name="accp", bufs=2))
        tp = ctx.enter_context(tc.tile_pool(name="tp", bufs=2))
        singles = ctx.enter_context(tc.tile_pool(name="singles", bufs=1))
        psum = ctx.enter_context(tc.tile_pool(name="psum", bufs=2, space="PSUM"))

        partial = singles.tile([128, n_loc * MC], f32)
        S_tiles = {}

        def emit_max(n, mc):
            S = S_tiles[(n, mc)]
            m5 = tree.tile([128, 5, W], bf16, tag="m5", name=f"m5_{n}_{mc}")
            nc.vector.tensor_max(m5[:], S[:, 0:5, :], S[:, 5:10, :])
            m2 = tree.tile([128, 2, W], bf16, tag="m2", name=f"m2_{n}_{mc}")
            nc.vector.tensor_max(m2[:], m5[:, 0:2, :], m5[:, 2:4, :])
            m = mpool.tile([128, W], bf16, tag="m", name=f"m_{n}_{mc}")
            nc.vector.tensor_max(m[:], m2[:, 0, :], m2[:, 1, :])
            nc.vector.tensor_max(m[:], m[:], m5[:, 4, :])
            nc.vector.tensor_max(m[:], m[:], S[:, 10, :])
            S_tiles[(n, mc, "m")] = m

        def emit_sub(n, mc):
            S = S_tiles[(n, mc)]
            m_ap = S_tiles[(n, mc, "m")][:]
            m_b = bass.AP(
                tensor=m_ap.tensor, offset=m_ap.offset,
                ap=[list(m_ap.ap[0]), [0, C], list(m_ap.ap[1])],
            )
            nc.vector.tensor_sub(S[:], S[:], m_b)

        def emit_expa(n, mc):
            S = S_tiles[(n, mc)]
            nc.scalar.activation(S[:, 0:6, :], S[:, 0:6, :], AF.Exp)

        def emit_expb(n, mc):
            S = S_tiles[(n, mc)]
            nc.scalar.activation(S[:, 6:11, :], S[:, 6:11, :], AF.Exp)

        def emit_sum(n, mc):
            S = S_tiles[(n, mc)]
            a5 = tree.tile([128, 5, W], bf16, tag="m5", name=f"a5_{n}_{mc}")
            nc.vector.tensor_add(a5[:], S[:, 0:5, :], S[:, 5:10, :])
            a2 = tree.tile([128, 2, W], bf16, tag="m2", name=f"a2_{n}_{mc}")
            nc.vector.tensor_add(a2[:], a5[:, 0:2, :], a5[:, 2:4, :])
            ac = mpool.tile([128, W], bf16, tag="m", name=f"ac_{n}_{mc}")
            nc.vector.tensor_add(ac[:], a2[:, 0, :], a2[:, 1, :])
            nc.vector.tensor_add(ac[:], ac[:], a5[:, 4, :])
            accf = accp.tile([128, W], f32, tag="accf", name=f"accf_{n}_{mc}")
            nc.vector.tensor_add(accf[:], ac[:], S[:, 10, :])
            S_tiles[(n, mc, "acc")] = accf

        def emit_recipstt(n, mc):
            accf = S_tiles[(n, mc, "acc")]
            t = tp.tile([128, W], f32, tag="t", name=f"t_{n}_{mc}")
            if RECIP_FAST:
                nc.vector.reciprocal_approx_fast(t[:], accf[:])
            else:
                nc.vector.reciprocal(t[:], accf[:])
            slot = n * MC + mc
            labt = labp.tile([128, W], bf16, tag="lab", name=f"lab_{n}_{mc}")
            nc.gpsimd.dma_start(labt[:], lab_d[n, mc * 128:(mc + 1) * 128, :])
            w_t = wpool.tile([128, W], f32, tag="w", name=f"w_{n}_{mc}")
            nc.vector.scalar_tensor_tensor(
                out=w_t[:], in0=labt[:], scalar=0.0, in1=t[:],
                op0=mybir.AluOpType.add, op1=mybir.AluOpType.mult,
                accum_out=partial[:, slot:slot + 1],
            )

        EMIT = {"max": emit_max, "sub": emit_sub, "expa": emit_expa,
                "expb": emit_expb, "sum": emit_sum, "recipstt": emit_recipstt}

        # chain(g, mc) pieces occupy the 4 slots of the next mc-pass (+1 spill)
        schedule = {}
        for g in range(n_loc):
            for mc in range(MC):
                dr = g * 12 + mc * 4 + 3    # last drain slot of this chunk
                for off, kind in ((1, "max"), (2, "sub"), (4, "expa"),
                                  (5, "expb"), (6, "sum"), (7, "recipstt")):
                    schedule.setdefault(dr + off, []).append((kind, g, mc))

        nc.vector.memset(partial[:], 0.0)

        plane_tiles = {}

        def load_pair(g, c):
            # one DMA instruction covers planes c and c+1 (if present)
            i = g * C + c
            npl = min(2, C - c)
            xt = lx.tile([128, npl, 4, 384], fp8, tag="xt", name=f"xt_{i}")
            nc.sync.dma_start(
                xt[:], xt2e_d[i:i + npl].rearrange("pl p s h -> p pl s h"))
            cE = lc.tile([128, npl, 6, 384], fp8, tag="cE", name=f"cE_{i}")
            nc.gpsimd.dma_start(
                cE[:], ccee_d[i:i + npl].rearrange("pl p s h -> p pl s h"))
            for j in range(npl):
                plane_tiles[(g, c + j)] = (xt, cE, j)

        TRIPLES = [(t0, min(3, C - t0)) for t0 in range(0, C, 3)]

        for c in range(0, C, 2):
            load_pair(0, c)
        for g in range(n_loc):
            for mc in range(MC):
                S_tiles[(g, mc)] = spool2.tile(
                    [128, C, W], bf16, tag="Sm", name=f"Sm_{g}_{mc}")
                for ti, (t0, L) in enumerate(TRIPLES):
                    ps = psum.tile([128, 3, 512], f32, tag="ps",
                                   name=f"ps_{g}_{mc}_{ti}")
                    for j in range(L):
                        xt, cE, pl = plane_tiles[(g, t0 + j)]
                        lhs1 = xt[:, pl, 0:2, mc * 128:(mc + 1) * 128]
                        lhs2 = xt[:, pl, 2:4, mc * 128:(mc + 1) * 128]
                        rhs1 = cE[:, pl, 0:2, :]
                        base = cE[:, pl, 2, :]
                        rhs2 = bass.AP(
                            tensor=base.tensor, offset=base.offset,
                            ap=[list(base.ap[0]), [(1 + mc) * 384, 2], [1, 384]],
                        )
                        nc.tensor.matmul(ps[:, j, 0:384], lhs1, rhs1,
                                         start=True, stop=False, perf_mode=DR)
                        nc.tensor.matmul(ps[:, j, 0:384], lhs2, rhs2,
                                         start=False, stop=True, perf_mode=DR)
                    nc.scalar.copy(S_tiles[(g, mc)][:, t0:t0 + L, :],
                                   ps[:, 0:L, 0:384])
                    sl = mc * 4 + ti        # slot within group, 0..11
                    if g + 1 < n_loc and sl < 6:
                        load_pair(g + 1, sl * 2)
                    s = g * 12 + sl
                    for kind, gg, mcc in schedule.pop(s, ()):
                        EMIT[kind](gg, mcc)

        pending = [schedule[s] for s in sorted(schedule)]
        queues = {}
        for lst in pending:
            for kind, gg, mcc in lst:
                queues.setdefault((gg, mcc), []).append((kind, gg, mcc))
        while any(queues.values()):
            for k in sorted(queues):
                if queues[k]:
                    kind, gg, mcc = queues[k].pop(0)
                    EMIT[kind](gg, mcc)

        pf = singles.tile([128, 1], f32)
        nc.vector.tensor_reduce(
            pf[:], partial[:], axis=mybir.AxisListType.X, op=mybir.AluOpType.add
        )
        nc.sync.dma_start(out_d[:, :], pf[:])

    nc.compile()
    return nc


def _get_compiled():
    global _COMPILED
    if _COMPILED is None:
        _COMPILED = _build()
    return _COMPILED


def _host_prep(x, centers, labels):
    x = np.asarray(x, dtype=np.float32)
    centers = np.asarray(centers, dtype=np.float32)
    labels_np = np.asarray(labels)

    n_zero = int((labels_np == 0).sum())

    xt2 = np.transpose(-2.0 * x, (0, 1, 3, 2)).astype(_FP8)   # (N, C, W, H)
    cc = centers.astype(_FP8)                                  # (N, C, H, W)
    ee = (x * x + centers * centers).astype(_FP8)              # (N, C, H, W)
    lab = labels_np.astype(np.float32).astype(_BF16)           # (N, H, W)
    ident = np.tile(np.eye(128, dtype=_FP8), (1, 3))           # (128, 384)

    # xt2e[i]: [128, 4, 384]: slabs 0..2 = xt2 k-chunks, slab 3 = [I,I,I]
    xt2e = np.empty((N, C, 128, 4, 384), dtype=_FP8)
    xt2_r = xt2.reshape(N, C, KC, 128, H)                      # k-chunks
    xt2e[:, :, :, 0:3, :] = np.transpose(xt2_r, (0, 1, 3, 2, 4))
    xt2e[:, :, :, 3, :] = ident[None, None]

    # ccee[i]: [128, 6, 384]: slabs 0..2 = centers k-chunks, 3..5 = ee chunks
    ccee = np.empty((N, C, 128, 6, 384), dtype=_FP8)
    cc_r = cc.reshape(N, C, KC, 128, W)
    ee_r = ee.reshape(N, C, MC, 128, W)
    ccee[:, :, :, 0:3, :] = np.transpose(cc_r, (0, 1, 3, 2, 4))
    ccee[:, :, :, 3:6, :] = np.transpose(ee_r, (0, 1, 3, 2, 4))

    in_maps = []
    for core in range(N_CORES):
        sl = slice(core * N_LOC, (core + 1) * N_LOC)
        in_maps.append(
            {
                "xt2e": np.ascontiguousarray(xt2e[sl]).reshape(PAIRS, 128, 4, 384),
                "ccee": np.ascontiguousarray(ccee[sl]).reshape(PAIRS, 128, 6, 384),
                "lab": np.ascontiguousarray(lab[sl]),
            }
        )
    return in_maps, n_zero


def kernel(x, centers, labels, _trace=False, _trace_kwargs=None):
    from concourse import bass_utils

    nc = _get_compiled()
    in_maps, n_zero = _host_prep(x, centers, labels)

    kwargs = {}
    if _trace:
        kwargs = dict(trace=True, **(_trace_kwargs or {}))
    res = bass_utils.run_bass_kernel_spmd(
        nc, in_maps, core_ids=list(range(N_CORES)), **kwargs
    )

    total = 0.0
    for core in range(N_CORES):
        total += float(res.results[core]["out"].astype(np.float64).sum())
    loss = (total + 1e-12 * n_zero) / float(N * H * W)
    out = np.float32(loss)
    if _trace:
        return out, res
    return out


# revision 3
# speedup vs baseline: 1.0241x; 1.0241x over previous
"""Trainium2 Bass kernel v2 for nn_CenterLossN.

Math (per batch n, class c; H=W=384, C=11, N=32):
    res[n,c]   = x[n,c]^2 + centers[n,c]^2 - 2 * x[n,c] @ centers[n,c]
    out[n,h,w] = 1 / sum_c exp(res_c - max_c res_c)
    loss       = sum(clip(out * labels, 1e-12, 1e12)) / (N*H*W)

v2 strategy (data-parallel over N across 8 cores, 4 batches/core):
  - fp8(e4m3) matmul in DoubleRow perf mode: per (plane, mc row-chunk) the
    contraction runs as 2 matmuls over 4 k-tiles: 3 tiles of (-2x)^T @ c
    plus a 4th tile identity @ ee injecting x^2+c^2 into PSUM. 768 PE
    cycles per chunk vs 1536 for the bf16 baseline.
  - host ships xt2e = [3 k-slabs of (-2x)^T | identity slab] fp8 and
    ccee = [3 k-slabs of centers | 3 mc-slabs of ee] fp8, partition-major.
  - mc-major plane order per batch-group; PSUM tile [128,3,512] f32
    (3 banks) per class-triplet, matmuls write bank-aligned [:, j, :384];
    one strided batched drain per triplet on ACT.
  - per-(n,mc) S tiles [128, C, W]; chain max->sub->exp->sum->recip->stt
    as 5-way trees / in-place ops on DVE+ACT (2x-rate 3-dim APs), emitted
    into later triplet slots (software pipelining); final two chunks run
    half-W chains interleaved to shorten the exposed tail.
  clip: only label==0 hits the 1e-12 floor; host adds 1e-12*count exactly.
"""

import numpy as np
import ml_dtypes

N, C, H, W = 32, 11, 384, 384
N_CORES = 8
N_LOC = N // N_CORES          # 4 batches per core
PAIRS = N_LOC * C             # 44 planes per core
MC = H // 128                 # 3 row-chunks
KC = W // 128                 # 3 contraction chunks

# feature flags resolved by optest on this deployment
RECIP_FAST = True     # vector.reciprocal_approx_fast works?
GPSIMD_OK = False     # gpsimd tensor ops work?

_BF16 = ml_dtypes.bfloat16
_FP8 = ml_dtypes.float8_e4m3
_COMPILED = None


def _build(n_loc=N_LOC):
    from contextlib import ExitStack
    import concourse.bass as bass
    import concourse.bacc as bacc
    import concourse.tile as tile
    from concourse import mybir

    bf16 = mybir.dt.bfloat16
    f32 = mybir.dt.float32
    fp8 = mybir.dt.float8e4
    AF = mybir.ActivationFunctionType
    DR = mybir.MatmulPerfMode.DoubleRow

    nc = bacc.Bacc("TRN2", target_bir_lowering=False, debug=False)

    pairs = n_loc * C
    xt2e_d = nc.dram_tensor("xt2e", [pairs, 128, 4, 384], fp8, kind="ExternalInput")
    ccee_d = nc.dram_tensor("ccee", [pairs, 128, 6, 384], fp8, kind="ExternalInput")
    lab_d = nc.dram_tensor("lab", [n_loc, H, W], bf16, kind="ExternalInput")
    out_d = nc.dram_tensor("out", [128, 1], f32, kind="ExternalOutput")

    with ExitStack() as ctx:
        tc = ctx.enter_context(tile.TileContext(nc))
        lx = ctx.enter_context(tc.tile_pool(name="lx", bufs=17))
        lc = ctx.enter_context(tc.tile_pool(name="lc", bufs=17))
        spool2 = ctx.enter_context(tc.tile_pool(name="spool2", bufs=6))
        labp = ctx.enter_context(tc.tile_pool(name="labp", bufs=2))
        tree = ctx.enter_context(tc.tile_pool(name="tree", bufs=2))
        mpool = ctx.enter_context(tc.tile_pool(name="mpool", bufs=4))
        wpool = ctx.enter_context(tc.tile_pool(name="wpool", bufs=2))
        accp = ctx.enter_context(tc.tile_pool(name="accp", bufs=2))
        tp = ctx.enter_context(tc.tile_pool(name="tp", bufs=2))
        singles = ctx.enter_context(tc.tile_pool(name="singles", bufs=1))
        psum = ctx.enter_context(tc.tile_pool(name="psum", bufs=2, space="PSUM"))

        partial = singles.tile([128, n_loc * MC], f32)
        S_tiles = {}

        def emit_max(n, mc):
            S = S_tiles[(n, mc)]
            m5 = tree.tile([128, 5, W], bf16, tag="m5", name=f"m5_{n}_{mc}")
            nc.vector.tensor_max(m5[:], S[:, 0:5, :], S[:, 5:10, :])
            m2 = tree.tile([128, 2, W], bf16, tag="m2", name=f"m2_{n}_{mc}")
            nc.vector.tensor_max(m2[:], m5[:, 0:2, :], m5[:, 2:4, :])
            m = mpool.tile([128, W], bf16, tag="m", name=f"m_{n}_{mc}")
            nc.vector.tensor_max(m[:], m2[:, 0, :], m2[:, 1, :])
            nc.vector.tensor_max(m[:], m[:], m5[:, 4, :])
            nc.vector.tensor_max(m[:], m[:], S[:, 10, :])
            S_tiles[(n, mc, "m")] = m

        def emit_sub(n, mc):
            S = S_tiles[(n, mc)]
            m_ap = S_tiles[(n, mc, "m")][:]
            m_b = bass.AP(
                tensor=m_ap.tensor, offset=m_ap.offset,
                ap=[list(m_ap.ap[0]), [0, C], list(m_ap.ap[1])],
            )
            nc.vector.tensor_sub(S[:], S[:], m_b)

        def emit_expa(n, mc):
            S = S_tiles[(n, mc)]
            nc.scalar.activation(S[:, 0:6, :], S[:, 0:6, :], AF.Exp)

        def emit_expb(n, mc):
            S = S_tiles[(n, mc)]
            nc.scalar.activation(S[:, 6:11, :], S[:, 6:11, :], AF.Exp)

        def emit_sum(n, mc):
            S = S_tiles[(n, mc)]
            a5 = tree.tile([128, 5, W], bf16, tag="m5", name=f"a5_{n}_{mc}")
            nc.vector.tensor_add(a5[:], S[:, 0:5, :], S[:, 5:10, :])
            a2 = tree.tile([128, 2, W], bf16, tag="m2", name=f"a2_{n}_{mc}")
            nc.vector.tensor_add(a2[:], a5[:, 0:2, :], a5[:, 2:4, :])
            ac = mpool.tile([128, W], bf16, tag="m", name=f"ac_{n}_{mc}")
            nc.vector.tensor_add(ac[:], a2[:, 0, :], a2[:, 1, :])
            nc.vector.tensor_add(ac[:], ac[:], a5[:, 4, :])
            accf = accp.tile([128, W], f32, tag="accf", name=f"accf_{n}_{mc}")
            nc.vector.tensor_add(accf[:], ac[:], S[:, 10, :])
            S_tiles[(n, mc, "acc")] = accf

        def emit_recipstt(n, mc):
            accf = S_tiles[(n, mc, "acc")]
            t = tp.tile([128, W], f32, tag="t", name=f"t_{n}_{mc}")
            if RECIP_FAST:
                nc.vector.reciprocal_approx_fast(t[:], accf[:])
            else:
                nc.vector.reciprocal(t[:], accf[:])
            slot = n * MC + mc
            labt = labp.tile([128, W], bf16, tag="lab", name=f"lab_{n}_{mc}")
            nc.gpsimd.dma_start(labt[:], lab_d[n, mc * 128:(mc + 1) * 128, :])
            w_t = wpool.tile([128, W], f32, tag="w", name=f"w_{n}_{mc}")
            nc.vector.scalar_tensor_tensor(
                out=w_t[:], in0=labt[:], scalar=0.0, in1=t[:],
                op0=mybir.AluOpType.add, op1=mybir.AluOpType.mult,
                accum_out=partial[:, slot:slot + 1],
            )

        def emit_maxh(n, mc, h):
            S = S_tiles[(n, mc)]
            ws = slice(h * 192, (h + 1) * 192)
            m5 = tree.tile([128, 5, 192], bf16, tag="m5h", name=f"m5h_{n}_{mc}_{h}")
            nc.vector.tensor_max(m5[:], S[:, 0:5, ws], S[:, 5:10, ws])
            m2 = tree.tile([128, 2, 192], bf16, tag="m2h", name=f"m2h_{n}_{mc}_{h}")
            nc.vector.tensor_max(m2[:], m5[:, 0:2, :], m5[:, 2:4, :])
            m = mpool.tile([128, 192], bf16, tag="mh", name=f"mh_{n}_{mc}_{h}")
            nc.vector.tensor_max(m[:], m2[:, 0, :], m2[:, 1, :])
            nc.vector.tensor_max(m[:], m[:], m5[:, 4, :])
            nc.vector.tensor_max(m[:], m[:], S[:, 10, ws])
            S_tiles[(n, mc, "mh", h)] = m

        def emit_subh(n, mc, h):
            S = S_tiles[(n, mc)]
            ws = slice(h * 192, (h + 1) * 192)
            m_ap = S_tiles[(n, mc, "mh", h)][:]
            m_b = bass.AP(
                tensor=m_ap.tensor, offset=m_ap.offset,
                ap=[list(m_ap.ap[0]), [0, C], list(m_ap.ap[1])],
            )
            nc.vector.tensor_sub(S[:, :, ws], S[:, :, ws], m_b)

        def emit_exph(n, mc, h):
            S = S_tiles[(n, mc)]
            ws = slice(h * 192, (h + 1) * 192)
            nc.scalar.activation(S[:, :, ws], S[:, :, ws], AF.Exp)

        def emit_sumh(n, mc, h):
            S = S_tiles[(n, mc)]
            ws = slice(h * 192, (h + 1) * 192)
            a5 = tree.tile([128, 5, 192], bf16, tag="m5h", name=f"a5h_{n}_{mc}_{h}")
            nc.vector.tensor_add(a5[:], S[:, 0:5, ws], S[:, 5:10, ws])
            a2 = tree.tile([128, 2, 192], bf16, tag="m2h", name=f"a2h_{n}_{mc}_{h}")
            nc.vector.tensor_add(a2[:], a5[:, 0:2, :], a5[:, 2:4, :])
            ac = mpool.tile([128, 192], bf16, tag="mh", name=f"ach_{n}_{mc}_{h}")
            nc.vector.tensor_add(ac[:], a2[:, 0, :], a2[:, 1, :])
            nc.vector.tensor_add(ac[:], ac[:], a5[:, 4, :])
            if h == 0:
                S_tiles[(n, mc, "acc")] = accp.tile(
                    [128, W], f32, tag="accf", name=f"accf_{n}_{mc}")
            accf = S_tiles[(n, mc, "acc")]
            nc.vector.tensor_add(accf[:, h * 192:(h + 1) * 192], ac[:],
                                 S[:, 10, ws])

        EMIT = {"max": emit_max, "sub": emit_sub, "expa": emit_expa,
                "expb": emit_expb, "sum": emit_sum, "recipstt": emit_recipstt,
                "maxh0": lambda n, mc: emit_maxh(n, mc, 0),
                "subh0": lambda n, mc: emit_subh(n, mc, 0),
                "exph0": lambda n, mc: emit_exph(n, mc, 0),
                "sumh0": lambda n, mc: emit_sumh(n, mc, 0),
                "maxh1": lambda n, mc: emit_maxh(n, mc, 1),
                "subh1": lambda n, mc: emit_subh(n, mc, 1),
                "exph1": lambda n, mc: emit_exph(n, mc, 1),
                "sumh1": lambda n, mc: emit_sumh(n, mc, 1)}

        # chain(g, mc) pieces occupy the 4 slots of the next mc-pass (+1 spill)
        schedule = {}
        for g in range(n_loc):
            for mc in range(MC):
                dr = g * 12 + mc * 4 + 3    # last drain slot of this chunk
                if g == n_loc - 1 and mc >= 1:
                    # final chunks: half-W chains pipeline ACT/DVE in the tail
                    kinds = ("maxh0", "subh0", "exph0", "maxh1", "subh1",
                             "sumh0", "exph1", "sumh1", "recipstt")
                    for off, kind in enumerate(kinds):
                        schedule.setdefault(dr + 1 + off, []).append(
                            (kind, g, mc))
                else:
                    for off, kind in ((1, "max"), (2, "sub"), (4, "expa"),
                                      (5, "expb"), (6, "sum"), (7, "recipstt")):
                        schedule.setdefault(dr + off, []).append((kind, g, mc))

        nc.vector.memset(partial[:], 0.0)

        plane_tiles = {}

        def load_pair(g, c, maxn=2, xq=None, cq=None):
            # one DMA instruction covers planes c .. c+maxn-1 (if present)
            i = g * C + c
            npl = min(maxn, C - c)
            xt = lx.tile([128, npl, 4, 384], fp8, tag="xt", name=f"xt_{i}")
            (xq or nc.sync).dma_start(
                xt[:], xt2e_d[i:i + npl].rearrange("pl p s h -> p pl s h"))
            cE = lc.tile([128, npl, 6, 384], fp8, tag="cE", name=f"cE_{i}")
            (cq or nc.gpsimd).dma_start(
                cE[:], ccee_d[i:i + npl].rearrange("pl p s h -> p pl s h"))
            for j in range(npl):
                plane_tiles[(g, c + j)] = (xt, cE, j)

        TRIPLES = [(t0, min(3, C - t0)) for t0 in range(0, C, 3)]

        for c in range(4):
            load_pair(0, c, maxn=1)
        for c in range(4, C, 2):
            load_pair(0, c)
        for g in range(n_loc):
            for mc in range(MC):
                S_tiles[(g, mc)] = spool2.tile(
                    [128, C, W], bf16, tag="Sm", name=f"Sm_{g}_{mc}")
                for ti, (t0, L) in enumerate(TRIPLES):
                    ps = psum.tile([128, 3, 512], f32, tag="ps",
                                   name=f"ps_{g}_{mc}_{ti}")
                    for j in range(L):
                        xt, cE, pl = plane_tiles[(g, t0 + j)]
                        lhs1 = xt[:, pl, 0:2, mc * 128:(mc + 1) * 128]
                        lhs2 = xt[:, pl, 2:4, mc * 128:(mc + 1) * 128]
                        rhs1 = cE[:, pl, 0:2, :]
                        base = cE[:, pl, 2, :]
                        rhs2 = bass.AP(
                            tensor=base.tensor, offset=base.offset,
                            ap=[list(base.ap[0]), [(1 + mc) * 384, 2], [1, 384]],
                        )
                        nc.tensor.matmul(ps[:, j, 0:384], lhs1, rhs1,
                                         start=True, stop=False, perf_mode=DR)
                        nc.tensor.matmul(ps[:, j, 0:384], lhs2, rhs2,
                                         start=False, stop=True, perf_mode=DR)
                    nc.scalar.copy(S_tiles[(g, mc)][:, t0:t0 + L, :],
                                   ps[:, 0:L, 0:384])
                    sl = mc * 4 + ti        # slot within group, 0..11
                    if g + 1 < n_loc and sl < 6:
                        load_pair(g + 1, sl * 2)
                    s = g * 12 + sl
                    for kind, gg, mcc in schedule.pop(s, ()):
                        EMIT[kind](gg, mcc)

        pending = [schedule[s] for s in sorted(schedule)]
        queues = {}
        for lst in pending:
            for kind, gg, mcc in lst:
                queues.setdefault((gg, mcc), []).append((kind, gg, mcc))
        while any(queues.values()):
            for k in sorted(queues):
                if queues[k]:
                    kind, gg, mcc = queues[k].pop(0)
                    EMIT[kind](gg, mcc)

        pf = singles.tile([128, 1], f32)
        nc.vector.tensor_reduce(
            pf[:], partial[:], axis=mybir.AxisListType.X, op=mybir.AluOpType.add
        )
        nc.sync.dma_start(out_d[:, :], pf[:])

    nc.compile()
    return nc


def _get_compiled():
    global _COMPILED
    if _COMPILED is None:
        _COMPILED = _build()
    return _COMPILED


def _host_prep(x, centers, labels):
    x = np.asarray(x, dtype=np.float32)
    centers = np.asarray(centers, dtype=np.float32)
    labels_np = np.asarray(labels)

    n_zero = int((labels_np == 0).sum())

    xt2 = np.transpose(-2.0 * x, (0, 1, 3, 2)).astype(_FP8)   # (N, C, W, H)
    cc = centers.astype(_FP8)                                  # (N, C, H, W)
    ee = (x * x + centers * centers).astype(_FP8)              # (N, C, H, W)
    lab = labels_np.astype(np.float32).astype(_BF16)           # (N, H, W)
    ident = np.tile(np.eye(128, dtype=_FP8), (1, 3))           # (128, 384)

    # xt2e[i]: [128, 4, 384]: slabs 0..2 = xt2 k-chunks, slab 3 = [I,I,I]
    xt2e = np.empty((N, C, 128, 4, 384), dtype=_FP8)
    xt2_r = xt2.reshape(N, C, KC, 128, H)                      # k-chunks
    xt2e[:, :, :, 0:3, :] = np.transpose(xt2_r, (0, 1, 3, 2, 4))
    xt2e[:, :, :, 3, :] = ident[None, None]

    # ccee[i]: [128, 6, 384]: slabs 0..2 = centers k-chunks, 3..5 = ee chunks
    ccee = np.empty((N, C, 128, 6, 384), dtype=_FP8)
    cc_r = cc.reshape(N, C, KC, 128, W)
    ee_r = ee.reshape(N, C, MC, 128, W)
    ccee[:, :, :, 0:3, :] = np.transpose(cc_r, (0, 1, 3, 2, 4))
    ccee[:, :, :, 3:6, :] = np.transpose(ee_r, (0, 1, 3, 2, 4))

    in_maps = []
    for core in range(N_CORES):
        sl = slice(core * N_LOC, (core + 1) * N_LOC)
        in_maps.append(
            {
                "xt2e": np.ascontiguousarray(xt2e[sl]).reshape(PAIRS, 128, 4, 384),
                "ccee": np.ascontiguousarray(ccee[sl]).reshape(PAIRS, 128, 6, 384),
                "lab": np.ascontiguousarray(lab[sl]),
            }
        )
    return in_maps, n_zero


def kernel(x, centers, labels, _trace=False, _trace_kwargs=None):
    from concourse import bass_utils

    nc = _get_compiled()
    in_maps, n_zero = _host_prep(x, centers, labels)

    kwargs = {}
    if _trace:
        kwargs = dict(trace=True, **(_trace_kwargs or {}))
    res = bass_utils.run_bass_kernel_spmd(
        nc, in_maps, core_ids=list(range(N_CORES)), **kwargs
    )

    total = 0.0
    for core in range(N_CORES):
        total += float(res.results[core]["out"].astype(np.float64).sum())
    loss = (total + 1e-12 * n_zero) / float(N * H * W)
    out = np.float32(loss)
    if _trace:
        return out, res
    return out
